# revision 1
# baseline (speedup 1.0000x reference)
"""Trainium2 Bass kernel: KMeans clustering loss (vq_codebook).

loss = mean_n min_k ||x_n - c_k||^2
  x = encode_output: [131072, 256] f32,  c = centroids: [1024, 256] f32.

Identity: min_k ||x-c_k||^2 = x_sq[n] + min_k (c_sq[k] - 2 x.c_k)
and the mean over n separates:  mean(x_sq) + mean(min_k(...)).

Data-parallel over N across 8 NeuronCores. Per core (16384 rows = 128
tiles of 128):
  PE  : cross = xT.T @ cnegT   (bf16 inputs, fp32 accum, [128,1024] PSUM)
  DVE : fused tensor_tensor_reduce: min_k(psum + csqB) -> [128,1]
  ACT : f32->bf16 cast of x;  Square with accum_out for per-row x_sq
  DMA : x tile loads + xbar transposes (contract dim onto partitions)
Output per core: [128, 2] partial sums (min-term, x_sq). Host combines.
"""

import sys

for _p in ("/opt/trn_rl_repo",):
    if _p not in sys.path:
        sys.path.insert(0, _p)

import numpy as np

N_FULL = 131072
D = 256
K = 1024
N_CORES = 8
N_CORE = N_FULL // N_CORES  # 16384
P = 128
NT = N_CORE // P  # 128 tiles per core


USE_XPOSE = True  # DMA-xbar transpose for x tiles (False: PE transpose)


def build_bass_program(n_core: int = N_CORE):
    import concourse.mybir as mybir
    from concourse.bacc import Bacc
    from concourse.masks import make_identity
    from concourse.tile import TileContext

    f32 = mybir.dt.float32
    bf16 = mybir.dt.bfloat16
    AF = mybir.ActivationFunctionType
    ALU = mybir.AluOpType

    NT = n_core // P

    nc = Bacc()

    x_dram = nc.dram_tensor("x", [n_core, D], f32, kind="ExternalInput")
    c_dram = nc.dram_tensor("c", [K, D], f32, kind="ExternalInput")
    out_dram = nc.dram_tensor("out", [P, 2], f32, kind="ExternalOutput")
    csq_scratch = nc.dram_tensor("csq_scratch", [P, K // P], f32, kind="Internal")

    KT = K // P  # 8 centroid tiles
    DCH = D // P  # 2 contract chunks

    with TileContext(nc) as tc:
        with (
            tc.tile_pool(name="persist", bufs=1) as persist,
            tc.tile_pool(name="cload", bufs=2) as cload,
            tc.tile_pool(name="cwork", bufs=2) as cwork,
            tc.tile_pool(name="xload", bufs=4) as xload,
            tc.tile_pool(name="xt0", bufs=4) as xtp0,
            tc.tile_pool(name="xt1", bufs=4) as xtp1,
            tc.tile_pool(name="x2s", bufs=2) as x2sp,
            tc.tile_pool(name="d2s", bufs=2) as d2sp,
            tc.tile_pool(name="psum", bufs=3, space="PSUM") as psump,
            tc.tile_pool(name="tpsum", bufs=2, space="PSUM") as tpsum,
        ):
            # ---- persistent tiles ----
            # cT[dch]: [128 d, 1024 k] bf16 holding (-2*c)^T chunk
            cT = [
                persist.tile([P, K], bf16, name=f"cT{d}", tag=f"cT{d}")
                for d in range(DCH)
            ]
            csq_rowF = persist.tile([1, K], f32, name="csq_rowF", tag="csq_rowF")
            csq_row = persist.tile([1, K], bf16, name="csq_row", tag="csq_row")
            ones_row = persist.tile([1, P], bf16, name="ones_row", tag="ones_row")
            csq_cols = persist.tile([P, KT], f32, name="csq_cols", tag="csq_cols")
            min_cols = persist.tile([P, NT], f32, name="min_cols", tag="min_cols")
            xsq_cols = persist.tile([P, NT], f32, name="xsq_cols", tag="xsq_cols")
            totals = persist.tile([P, 2], f32, name="totals", tag="totals")
            ident = persist.tile([P, P], f32, name="ident", tag="ident")
            make_identity(nc, ident[:])

            # ---- setup: centroid prep ----
            for j in range(KT):
                cF = cload.tile([P, D], f32, tag="cF")
                nc.sync.dma_start(cF[:], c_dram[j * P : (j + 1) * P, :])
                # c_sq row sums (fp32, exact) for this k-tile
                c2 = cwork.tile([P, D], f32, tag="c2")
                nc.scalar.activation(
                    c2[:], cF[:], AF.Square, accum_out=csq_cols[:, j : j + 1]
                )
                # PE-transpose each f32 chunk; scale by -2 and cast to bf16
                # on the way out of PSUM
                for dch in range(DCH):
                    pt = tpsum.tile([P, P], f32, tag="pt")
                    nc.tensor.transpose(
                        pt[:], cF[:, dch * P : (dch + 1) * P], ident[:]
                    )
                    nc.vector.tensor_scalar_mul(
                        cT[dch][:, j * P : (j + 1) * P], pt[:], -2.0
                    )

            # csq_cols [128, 8] -> csq row [1, 1024] via DRAM bounce
            # (k = j*128 + p  ->  csq_scratch[p, j] read back in (j p) order)
            nc.sync.dma_start(csq_scratch[:, :], csq_cols[:])
            csq_row_src = csq_scratch[:, :].rearrange("p j -> j p")[None, :, :]
            nc.sync.dma_start(
                csq_rowF[0:1, :].rearrange("o (j p) -> o j p", j=KT), csq_row_src
            )
            nc.vector.tensor_copy(csq_row[:], csq_rowF[:])
            nc.vector.memset(ones_row[:], 1.0)

            # ---- main loop over 128-row tiles ----
            for t in range(NT):
                xF = xload.tile([P, D], f32, tag="xF")
                nc.sync.dma_start(xF[:], x_dram[t * P : (t + 1) * P, :])

                x2 = x2sp.tile([P, D], f32, tag="x2")
                nc.scalar.activation(
                    x2[:], xF[:], AF.Square, accum_out=xsq_cols[:, t : t + 1]
                )

                xT0 = xtp0.tile([P, P], bf16, tag="xT0")
                xT1 = xtp1.tile([P, P], bf16, tag="xT1")
                if USE_XPOSE:
                    # cast f32->bf16 on ACT, transpose via DMA xbar
                    xB = d2sp.tile([P, D], bf16, tag="xB")
                    nc.scalar.copy(xB[:], xF[:])
                    nc.sync.dma_start_transpose(xT0[:], xB[:, 0:P])
                    nc.sync.dma_start_transpose(xT1[:], xB[:, P : 2 * P])
                else:
                    # PE-transpose f32 chunks; ACT casts PSUM->SBUF bf16
                    for dch, xTc in enumerate((xT0, xT1)):
                        pt = tpsum.tile([P, P], f32, tag="pt")
                        nc.tensor.transpose(
                            pt[:], xF[:, dch * P : (dch + 1) * P], ident[:]
                        )
                        nc.scalar.copy(xTc[:], pt[:])

                ps = psump.tile([P, K], f32, tag="ps")
                # init both PSUM banks with c_sq via a 1-row ones matmul,
                # then accumulate -2*cross on top
                for h in range(2):
                    nc.tensor.matmul(
                        ps[:, h * 512 : (h + 1) * 512],
                        lhsT=ones_row[0:1, :],
                        rhs=csq_row[0:1, h * 512 : (h + 1) * 512],
                        start=True,
                        stop=False,
                    )
                for dch, xTc in enumerate((xT0, xT1)):
                    for h in range(2):
                        nc.tensor.matmul(
                            ps[:, h * 512 : (h + 1) * 512],
                            lhsT=xTc[:],
                            rhs=cT[dch][:, h * 512 : (h + 1) * 512],
                            start=False,
                            stop=(dch == DCH - 1),
                        )

                # d2 tile now complete in PSUM: min over k
                nc.vector.tensor_reduce(
                    min_cols[:, t : t + 1],
                    ps[:],
                    axis=mybir.AxisListType.X,
                    op=ALU.min,
                )

            # ---- epilogue ----
            nc.vector.reduce_sum(
                totals[:, 0:1], min_cols[:], axis=mybir.AxisListType.X
            )
            nc.vector.reduce_sum(
                totals[:, 1:2], xsq_cols[:], axis=mybir.AxisListType.X
            )
            nc.sync.dma_start(out_dram[:, :], totals[:])

    nc.finalize()
    return nc


_NC_CACHE = None


def _get_program():
    global _NC_CACHE
    if _NC_CACHE is None:
        _NC_CACHE = build_bass_program()
    return _NC_CACHE


def kernel(encode_output: np.ndarray, centroids: np.ndarray) -> np.ndarray:
    from concourse.bass_utils import run_bass_kernel_spmd

    x = np.ascontiguousarray(np.asarray(encode_output, dtype=np.float32))
    c = np.ascontiguousarray(np.asarray(centroids, dtype=np.float32))
    assert x.shape == (N_FULL, D) and c.shape == (K, D)

    nc = _get_program()
    in_maps = [
        {"x": x[i * N_CORE : (i + 1) * N_CORE], "c": c} for i in range(N_CORES)
    ]
    res = run_bass_kernel_spmd(nc, in_maps, core_ids=list(range(N_CORES)))
    total = np.float64(0.0)
    for r in res.results:
        total += r["out"].astype(np.float64).sum()
    return np.asarray(total / N_FULL, dtype=np.float32)


if __name__ == "__main__":
    rng = np.random.default_rng(0)
    x = rng.standard_normal((N_FULL, D), dtype=np.float32)
    c = rng.standard_normal((K, D), dtype=np.float32)
    print("kernel:", kernel(x, c))



# revision 16
# speedup vs baseline: 2.4627x; 2.4627x over previous
"""Trainium2 Bass kernel: KMeans clustering loss (vq_codebook).

loss = mean_n min_k ||x_n - c_k||^2
  x = encode_output: [131072, 256] f32,  c = centroids: [1024, 256] f32.

Identity: min_k ||x-c_k||^2 = x_sq[n] + min_k (c_sq[k] - 2 x.c_k)
and the mean over n separates:  mean(x_sq) + mean(min_k(...)).

Data-parallel over N across 8 NeuronCores; per core 16384 rows.

Per-core pipeline (groups of 512 rows = 4 tiles of 128):
  DMA(act q) : x group load [128, 4, 256] f32 -- partition p holds rows
               4p..4p+3 of the group (contiguous 4KB packets).
  ACT        : Square+accum (x_sq), f32->fp8e4 cast.
  DMA(sync q): one batched 16-bit xbar transpose of the fp8 PAIR view
               -> xT[ki, s, n] holding d-pairs (2ki, 2ki+1) per cell:
               exactly the DoubleRow interleaved lhsT layout.
  PE         : per tile: 2 fp16 ones x csq_row matmuls init the two PSUM
               half-banks with c_sq, then 2 fp8 DoubleRow matmuls (full
               d=256 contract each) accumulate -2 x.c on top.
  DVE        : fused tensor_tensor_reduce: min(psA, psB) -> min-reduce
               -> min_cols[:, tile]  (one op per tile).
Output per core: [128, 2] partial sums (min-term, x_sq). Host combines.
"""

import sys

for _p in ("/opt/trn_rl_repo",):
    if _p not in sys.path:
        sys.path.insert(0, _p)

import numpy as np

N_FULL = 131072
D = 256
K = 1024
N_CORES = 8
N_CORE = N_FULL // N_CORES  # 16384
P = 128
R = 4  # rows per partition per group (= tiles per group)
G_ROWS = P * R  # 512
NT = N_CORE // P  # 128 tiles per core

# tensor_tensor_reduce (custom DVE-table op) wedges the device under this
# runtime, and dual-PSUM tensor_tensor is rejected by the BIR verifier.
# So: c_sq goes into PSUM via fp16 ones x csq_row init matmuls, and the
# min is a plain vector.tensor_reduce over PSUM (both baseline-verified).


def build_bass_program(n_core: int = N_CORE):
    import concourse.mybir as mybir
    from concourse.bacc import Bacc
    from concourse.tile import TileContext

    f32 = mybir.dt.float32
    f16 = mybir.dt.float16
    bf16 = mybir.dt.bfloat16
    fp8 = mybir.dt.float8e4
    u16 = mybir.dt.uint16
    AF = mybir.ActivationFunctionType
    ALU = mybir.AluOpType

    NG = n_core // G_ROWS  # groups of 512 rows
    nt = n_core // P

    KT = K // P  # 8 centroid tiles

    nc = Bacc()

    x_dram = nc.dram_tensor("x", [n_core, D], f32, kind="ExternalInput")
    c_dram = nc.dram_tensor("c", [K, D], f32, kind="ExternalInput")
    out_dram = nc.dram_tensor("out", [P, 2], f32, kind="ExternalOutput")
    csq_scratch = nc.dram_tensor("csq_scratch", [P, KT], f32, kind="Internal")

    with TileContext(nc) as tc:
        with (
            tc.tile_pool(name="persist", bufs=1) as persist,
            tc.tile_pool(name="cload", bufs=2) as cload,
            tc.tile_pool(name="cwork", bufs=2) as cwork,
            tc.tile_pool(name="xload", bufs=3) as xload,
            tc.tile_pool(name="x8p", bufs=3) as x8p,
            tc.tile_pool(name="xtp", bufs=3) as xtp,
            tc.tile_pool(name="sqp", bufs=2) as sqp,
            tc.tile_pool(name="psum", bufs=3, space="PSUM") as psump,
        ):
            # ---- persistent tiles ----
            # cT8u16[ki, j, k]: 16-bit view of fp8 pairs (-2c[k, 2ki+ko])
            cT8u16 = persist.tile([P, KT, P], u16, name="cT8u16", tag="cT8u16")
            # plane-separated DoubleRow ifmap: [ki, ko, k] = -2c[k, 2ki+ko]
            cT8sep = persist.tile([P, 2, K], fp8, name="cT8sep", tag="cT8sep")
            csq_rowF = persist.tile([1, K], f32, name="csq_rowF", tag="csq_rowF")
            csq_row16 = persist.tile([1, K], f16, name="csq_row16", tag="csq_row16")
            ones_row = persist.tile([1, P], f16, name="ones_row", tag="ones_row")
            csq_cols = persist.tile([P, KT], f32, name="csq_cols", tag="csq_cols")
            min_cols = persist.tile([P, nt], f32, name="min_cols", tag="min_cols")
            xsq_cols = persist.tile([P, NG], f32, name="xsq_cols", tag="xsq_cols")
            totals = persist.tile([P, 2], f32, name="totals", tag="totals")

            # ---- setup: centroid prep ----
            for j in range(KT):
                cF = cload.tile([P, D], f32, tag="cF")
                nc.sync.dma_start(cF[:], c_dram[j * P : (j + 1) * P, :])
                # c_sq row sums (fp32, exact) for this k-tile
                c2 = cwork.tile([P, D], f32, tag="c2")
                nc.scalar.activation(
                    c2[:], cF[:], AF.Square, accum_out=csq_cols[:, j : j + 1]
                )
                # -2c cast to fp8; pair-transpose the 16-bit view so that
                # partition ki holds d-pair (2ki, 2ki+1) for all k.
                c8 = cwork.tile([P, D], fp8, tag="c8")
                nc.scalar.mul(c8[:], cF[:], -2.0)
                nc.sync.dma_start_transpose(
                    cT8u16[:, j, :], c8[:].bitcast(u16)
                )

            # csq_cols [128, 8] -> csq row [1, 1024] via DRAM bounce
            # (k = j*128 + p  ->  csq_scratch[p, j] read back in (j p) order)
            nc.sync.dma_start(csq_scratch[:, :], csq_cols[:])
            csq_row_src = csq_scratch[:, :].rearrange("p j -> j p")[None, :, :]
            nc.sync.dma_start(
                csq_rowF[0:1, :].rearrange("o (j p) -> o j p", j=KT), csq_row_src
            )
            nc.vector.tensor_copy(csq_row16[:], csq_rowF[:])
            nc.vector.memset(ones_row[:], 1.0)
            # de-interleave pair-transposed c into plane-separated layout
            nc.vector.tensor_copy(
                cT8sep[:].rearrange("a two (j k) -> a two j k", j=KT),
                cT8u16[:].bitcast(fp8).rearrange("a j (k two) -> a two j k", two=2),
            )


            # ---- main loop over groups of 512 rows ----
            for g in range(NG):
                xF = xload.tile([P, R, D], f32, tag="xF")
                # partition p <- rows g*512 + 4p + s (4KB contiguous per line)
                nc.scalar.dma_start(
                    xF[:],
                    x_dram[g * G_ROWS : (g + 1) * G_ROWS, :].rearrange(
                        "(p s) d -> p s d", s=R
                    ),
                )

                x2 = sqp.tile([P, R * D], f32, tag="x2")
                nc.scalar.activation(
                    x2[:],
                    xF[:].rearrange("p s d -> p (s d)"),
                    AF.Square,
                    accum_out=xsq_cols[:, g : g + 1],
                )

                x8 = x8p.tile([P, R, D], fp8, tag="x8")
                nc.scalar.copy(x8[:], xF[:])

                # one batched xbar transpose: in [128, R*128] u16 pairs ->
                # out[ki, s, p] = pair(x row 4p+s, dims 2ki..2ki+1)
                xT = xtp.tile([P, R, P], u16, tag="xT")
                nc.sync.dma_start_transpose(
                    xT[:], x8[:].bitcast(u16).rearrange("p s k -> p (s k)")
                )
                xT8 = xT[:].bitcast(fp8)  # [128, R, 256]

                for s in range(R):
                    t_idx = g * R + s
                    ps = psump.tile([P, K], f32, tag="ps")
                    # SwInterleave weights: A/B pairs adjacent per column,
                    # columns reversed -> psum rows permuted (n -> 127-n),
                    # harmless since downstream only sums row-wise mins.
                    lhsT = xT8[:, s, :]
                    # init both PSUM half-banks with c_sq (fp16, exact-ish)
                    for h in range(2):
                        nc.tensor.matmul(
                            ps[:, h * 512 : (h + 1) * 512],
                            lhsT=ones_row[0:1, :],
                            rhs=csq_row16[0:1, h * 512 : (h + 1) * 512],
                            start=True,
                            stop=False,
                        )
                    # fp8 DoubleRow: full d=256 contract in one matmul per half
                    for h in range(2):
                        rhs = cT8sep[:, :, h * 512 : (h + 1) * 512]
                        nc.tensor.matmul(
                            ps[:, h * 512 : (h + 1) * 512],
                            lhsT=lhsT,
                            rhs=rhs,
                            start=False,
                            stop=True,
                            perf_mode=mybir.MatmulPerfMode.DoubleRowSwInterleave,
                        )

                    nc.vector.tensor_reduce(
                        min_cols[:, t_idx : t_idx + 1],
                        ps[:],
                        axis=mybir.AxisListType.X,
                        op=ALU.min,
                    )

            # ---- epilogue ----
            nc.vector.reduce_sum(
                totals[:, 0:1], min_cols[:], axis=mybir.AxisListType.X
            )
            nc.vector.reduce_sum(
                totals[:, 1:2], xsq_cols[:], axis=mybir.AxisListType.X
            )
            nc.sync.dma_start(out_dram[:, :], totals[:])

    nc.finalize()
    return nc


_NC_CACHE = None


def _get_program():
    global _NC_CACHE
    if _NC_CACHE is None:
        _NC_CACHE = build_bass_program()
    return _NC_CACHE


def kernel(encode_output: np.ndarray, centroids: np.ndarray) -> np.ndarray:
    from concourse.bass_utils import run_bass_kernel_spmd

    x = np.ascontiguousarray(np.asarray(encode_output, dtype=np.float32))
    c = np.ascontiguousarray(np.asarray(centroids, dtype=np.float32))
    assert x.shape == (N_FULL, D) and c.shape == (K, D)

    nc = _get_program()
    in_maps = [
        {"x": x[i * N_CORE : (i + 1) * N_CORE], "c": c} for i in range(N_CORES)
    ]
    res = run_bass_kernel_spmd(nc, in_maps, core_ids=list(range(N_CORES)))
    total = np.float64(0.0)
    for r in res.results:
        total += r["out"].astype(np.float64).sum()
    return np.asarray(total / N_FULL, dtype=np.float32)


if __name__ == "__main__":
    rng = np.random.default_rng(0)
    x = rng.standard_normal((N_FULL, D), dtype=np.float32)
    c = rng.standard_normal((K, D), dtype=np.float32)
    print("kernel:", kernel(x, c))
